# revision 1
# baseline (speedup 1.0000x reference)
"""BasisExpansionLayer Trainium2 kernel.

Full input x: [256, 512] f32. Full output: [256, 512 + 512*512 + 512] f32
laid out as [sin(x) | (x_i * x_j for the cartesian i,j grid) | x].

Sharding: the outer-product index i (512 values) is split across the 8
cores, 64 i-values each.  Every core holds the full batch (256 rows ->
2 x 128 SBUF partitions), so the DVE/ACT engines run with all 128 lanes
busy and both multiply operands are direct slices of the loaded x tiles
(no transposes, no cross-partition broadcasts, no communication).

Per core:
  pair_out[b, i_local*512 + j] = x[b, j] * x[b, c*64 + i_local]
  sin_out[b, i_local]          = sin(x[b, c*64 + i_local])
  id_out[b, i_local]           = x[b, c*64 + i_local]
The host reassembles the full [256, 263168] output from the 8 shards.
"""

import numpy as np

B = 256
D = 512
M = 8            # cores
IPC = D // M     # 64 i-values per core

_CACHE = {}

# sin(x) = y * p(y^2), y = x - round(x/2pi)*2pi (Cody-Waite), |y| <= pi.
# p coeffs: IRLS-minimax fit, end-to-end fp32 max abs err 5.3e-7.
SIN_COEFFS = [
    1.0,
    -0.166666641831398,
    0.00833331048488617,
    -0.0001984015543712303,
    2.752945647443994e-06,
    -2.467699466990325e-08,
    1.345159122978501e-10,
]
INV2PI = 0.15915494309189535
MAGIC = 12582912.0  # 1.5 * 2**23: fp32 round-to-nearest via add/sub
TWOPI_HI = 6.28125
TWOPI_LO = 0.0019353071795864769

# knobs: warm_plan = small ramp store tiles (own pool, own slot size),
# tile_plan = steady-state store-tile sizes, act_every = every
# act_every-th multiply goes to the scalar engine.
DEFAULT_CFG = dict(
    warm_plan=(2, 4, 6, 10),  # ramp store tiles for half 0 (own small pool)
    tile_plan=(16, 26),       # steady-state store tiles for half 0
    tile_plan2=(32, 32),      # half 1: pipeline already full, no ramp
    act_every=4,
    bufs=2,
    warm_bufs=3,
    repeat=1,
    sin_pos=4,       # emit the sin chain after this many pair tiles
    split_load=True,  # issue half-1 loads on the ACT HWDGE ring
    sin_from_sbuf=True,  # build the sin input by DVE copy, not DMA reload
)


def _build_nc(cfg=None):
    import concourse.bass as bass  # noqa: F401
    import concourse.mybir as mybir
    import concourse.tile as tile
    from concourse import bacc

    cfg = {**DEFAULT_CFG, **(cfg or {})}
    warm_plan = tuple(cfg.get("warm_plan") or ())
    tile_plan = tuple(cfg["tile_plan"])
    tile_plan2 = tuple(cfg.get("tile_plan2") or ())
    act_every = cfg["act_every"]
    bufs = cfg["bufs"]
    warm_bufs = cfg.get("warm_bufs", 3)
    repeat = cfg["repeat"]
    sin_pos = cfg.get("sin_pos", 1 if cfg.get("sin_late") else 0)
    split_load = cfg.get("split_load", False)
    sin_from_sbuf = cfg.get("sin_from_sbuf", False)
    store_ring = cfg.get("store_ring", "sp")  # "sp" | "alt"
    warm_alt = cfg.get("warm_alt", False)  # whole warm tiles alternate DVE/ACT
    # per-half store plans: (size, use_warm_pool) lists
    plan_h0 = [(g, True) for g in warm_plan] + [(g, False) for g in tile_plan]
    plan_h1 = (
        [(g, False) for g in tile_plan2] if tile_plan2 else list(plan_h0)
    )
    assert sum(g for g, _ in plan_h0) == IPC, plan_h0
    assert sum(g for g, _ in plan_h1) == IPC, plan_h1

    f32 = mybir.dt.float32
    nc = bacc.Bacc("TRN2", target_bir_lowering=False, debug=False, num_devices=M)

    x = nc.dram_tensor("x", [B, D], f32, kind="ExternalInput")
    xs = nc.dram_tensor("xs", [B, IPC], f32, kind="ExternalInput")
    # sin and identity merged into one [B, 2*IPC] tensor: per-partition DMA
    # runs of 512 B (the SDMA line-rate minimum) instead of 2x256 B RMW.
    sid_out = nc.dram_tensor("sid_out", [B, 2 * IPC], f32, kind="ExternalOutput")
    pair_out = nc.dram_tensor("pair_out", [B, IPC * D], f32, kind="ExternalOutput")

    with tile.TileContext(nc) as tc:
        with (
            tc.tile_pool(name="xp", bufs=1) as xpool,
            tc.tile_pool(name="sp", bufs=1) as spool,
            tc.tile_pool(name="wp", bufs=warm_bufs) as wpool,
            tc.tile_pool(name="op", bufs=bufs) as opool,
        ):
            alu = mybir.AluOpType
            for _rep in range(repeat):
                xt = []
                xst = []
                for h in range(2):
                    rows = slice(h * 128, (h + 1) * 128)
                    # half-1 loads go out on the ACT HWDGE ring so both
                    # rings generate descriptors in parallel at t=0.
                    dma_eng = nc.scalar if (split_load and h == 1) else nc.sync
                    ts = xpool.tile([128, IPC], f32, tag=f"xs{h}")
                    dma_eng.dma_start(ts[:], xs[rows, :])
                    xst.append(ts)
                    t = xpool.tile([128, D], f32, tag=f"x{h}")
                    dma_eng.dma_start(t[:], x[rows, :])
                    xt.append(t)

                def emit_sin():
                    # accurate sin via odd polynomial on DVE: both batch
                    # halves side by side in one [128, 2*IPC] tile.
                    W = 2 * IPC
                    xsin = spool.tile([128, W], f32, tag="xsin")
                    for h in range(2):
                        if sin_from_sbuf:
                            nc.vector.tensor_copy(
                                xsin[:, h * IPC : (h + 1) * IPC], xst[h][:]
                            )
                        else:
                            nc.sync.dma_start(
                                xsin[:, h * IPC : (h + 1) * IPC],
                                xs[h * 128 : (h + 1) * 128, :],
                            )
                    tt = spool.tile([128, W], f32, tag="t")
                    # t = x*inv2pi + magic ; k = t - magic (round-to-nearest)
                    nc.vector.tensor_scalar(
                        out=tt[:], in0=xsin[:], scalar1=INV2PI, scalar2=MAGIC,
                        op0=alu.mult, op1=alu.add,
                    )
                    kk = spool.tile([128, W], f32, tag="k")
                    nc.vector.tensor_scalar_sub(kk[:], tt[:], MAGIC)
                    # y = (x - k*2pi_hi) - k*2pi_lo
                    kh = spool.tile([128, W], f32, tag="kh")
                    nc.vector.tensor_scalar_mul(kh[:], kk[:], TWOPI_HI)
                    yy = spool.tile([128, W], f32, tag="y")
                    nc.vector.tensor_sub(yy[:], xsin[:], kh[:])
                    kl = spool.tile([128, W], f32, tag="kl")
                    nc.vector.tensor_scalar_mul(kl[:], kk[:], TWOPI_LO)
                    nc.vector.tensor_sub(yy[:], yy[:], kl[:])
                    uu = spool.tile([128, W], f32, tag="u")
                    nc.vector.tensor_mul(uu[:], yy[:], yy[:])
                    # Horner: p = (((c6*u + c5)*u + c4)...)*u + c0
                    pp = spool.tile([128, W], f32, tag="p")
                    nc.vector.tensor_scalar(
                        out=pp[:], in0=uu[:], scalar1=SIN_COEFFS[6],
                        scalar2=SIN_COEFFS[5], op0=alu.mult, op1=alu.add,
                    )
                    for cidx in (4, 3, 2, 1, 0):
                        nc.vector.tensor_mul(pp[:], pp[:], uu[:])
                        nc.vector.tensor_scalar_add(
                            pp[:], pp[:], SIN_COEFFS[cidx]
                        )
                    for h in range(2):
                        rows = slice(h * 128, (h + 1) * 128)
                        sid = spool.tile([128, W], f32, tag=f"sid{h}")
                        sl = slice(h * IPC, (h + 1) * IPC)
                        # final Horner multiply lands directly in the merged
                        # tile; identity columns are a DVE copy of xs.
                        nc.vector.tensor_mul(sid[:, 0:IPC], pp[:, sl], yy[:, sl])
                        nc.vector.tensor_copy(sid[:, IPC : 2 * IPC], xst[h][:])
                        nc.sync.dma_start(sid_out[rows, :], sid[:])

                if sin_pos == 0:
                    emit_sin()

                # pair part: out[p, k*512 + j] = x[p, j] * x[p, i]
                n_op = 0
                n_tile = 0
                for h in range(2):
                    rows = slice(h * 128, (h + 1) * 128)
                    i0 = 0
                    for g_sz, warm in (plan_h0 if h == 0 else plan_h1):
                        pool = wpool if warm else opool
                        ot = pool.tile(
                            [128, g_sz * D], f32, tag="warm" if warm else "out"
                        )
                        for k in range(g_sz):
                            i = i0 + k
                            dst = ot[:, k * D : (k + 1) * D]
                            scal = xst[h][:, i : i + 1]
                            if warm_alt and warm:
                                on_act = n_tile % 2 == 1
                            else:
                                on_act = n_op % act_every == act_every - 1
                            if on_act:
                                # ACT: out = in * scale (activation Copy)
                                nc.scalar.mul(dst, xt[h][:], scal)
                            else:
                                nc.vector.tensor_scalar_mul(dst, xt[h][:], scal)
                            n_op += 1
                        st_eng = (
                            nc.scalar
                            if (store_ring == "alt" and n_tile % 2 == 1)
                            else nc.sync
                        )
                        st_eng.dma_start(
                            pair_out[rows, i0 * D : (i0 + g_sz) * D], ot[:]
                        )
                        i0 += g_sz
                        n_tile += 1
                        if n_tile == sin_pos:
                            emit_sin()
    nc.compile()
    return nc


def _get_nc(cfg=None):
    key = repr(cfg)
    if key not in _CACHE:
        _CACHE[key] = _build_nc(cfg)
    return _CACHE[key]


def _in_maps(x):
    x = np.ascontiguousarray(np.asarray(x, dtype=np.float32))
    assert x.shape == (B, D)
    return [
        {
            "x": x,
            "xs": np.ascontiguousarray(x[:, c * IPC : (c + 1) * IPC]),
        }
        for c in range(M)
    ]


def _get_exec(cfg=None):
    """Build the 8-core sharded PJRT callable once per process.

    Mirrors bass2jax.run_bass_via_pjrt's multi-core path, but caches the
    jitted executable: loading/executing a second NEFF in the same process
    can wedge the exec unit, while re-executing one cached executable with
    donated output buffers is reliable.
    """
    key = ("exec", repr(cfg))
    if key in _CACHE:
        return _CACHE[key]

    import jax
    from jax.sharding import Mesh, PartitionSpec
    from jax.experimental.shard_map import shard_map
    import concourse.mybir as mybir
    from concourse import bass2jax

    nc = _get_nc(cfg)
    bass2jax.install_neuronx_cc_hook()

    partition_name = nc.partition_id_tensor.name if nc.partition_id_tensor else None
    in_names, out_names, out_avals, out_shapes = [], [], [], []
    for alloc in nc.m.functions[0].allocations:
        if not isinstance(alloc, mybir.MemoryLocationSet):
            continue
        name = alloc.memorylocations[0].name
        if alloc.kind == "ExternalInput":
            if name != partition_name:
                in_names.append(name)
        elif alloc.kind == "ExternalOutput":
            shape = tuple(alloc.tensor_shape)
            dtype = mybir.dt.np(alloc.dtype)
            out_names.append(name)
            out_avals.append(jax.core.ShapedArray(shape, dtype))
            out_shapes.append((shape, dtype))
    n_params = len(in_names)
    n_outs = len(out_avals)
    all_in_names = list(in_names) + list(out_names)
    if partition_name is not None:
        all_in_names.append(partition_name)

    def _body(*args):
        operands = list(args)
        if partition_name is not None:
            operands.append(bass2jax.partition_id_tensor())
        return tuple(
            bass2jax._bass_exec_p.bind(
                *operands,
                out_avals=tuple(out_avals),
                in_names=tuple(all_in_names),
                out_names=tuple(out_names),
                lowering_input_output_aliases=(),
                sim_require_finite=True,
                sim_require_nnan=True,
                nc=nc,
            )
        )

    devices = jax.devices()[:M]
    assert len(devices) == M, f"need {M} NeuronCores, found {len(devices)}"
    mesh = Mesh(np.asarray(devices), ("core",))
    in_specs = (PartitionSpec("core"),) * (n_params + n_outs)
    out_specs = (PartitionSpec("core"),) * n_outs
    donate = tuple(range(n_params, n_params + n_outs))
    sharded = jax.jit(
        shard_map(_body, mesh=mesh, in_specs=in_specs, out_specs=out_specs,
                  check_rep=False),
        donate_argnums=donate,
        keep_unused=True,
    )

    def run(in_maps):
        concat_in = [
            np.concatenate([np.asarray(in_maps[c][n]) for c in range(M)], axis=0)
            for n in in_names
        ]
        concat_zeros = [
            np.zeros((M * s[0], *s[1:]), dt) for s, dt in out_shapes
        ]
        outs = sharded(*concat_in, *concat_zeros)
        return [
            {
                name: np.asarray(outs[i]).reshape(M, *out_shapes[i][0])[c]
                for i, name in enumerate(out_names)
            }
            for c in range(M)
        ]

    _CACHE[key] = run
    return run


def _run(x, cfg=None):
    from concourse._compat import axon_active

    if axon_active():
        return _get_exec(cfg)(_in_maps(x))
    # native NRT path (no axon): run_bass_kernel_spmd handles the NEFF
    # load/exec/unload lifecycle per call.
    from concourse import bass_utils

    res = bass_utils.run_bass_kernel_spmd(
        _get_nc(cfg), _in_maps(x), core_ids=list(range(M))
    )
    return res.results


def kernel(**inputs):
    results = _run(inputs["x"])
    out = np.empty((B, 2 * D + D * D), dtype=np.float32)
    for c in range(M):
        r = results[c]
        out[:, c * IPC : (c + 1) * IPC] = r["sid_out"][:, :IPC]
        out[:, D + c * IPC * D : D + (c + 1) * IPC * D] = r["pair_out"]
        out[:, D + D * D + c * IPC : D + D * D + (c + 1) * IPC] = r["sid_out"][:, IPC:]
    return out



# revision 11
# speedup vs baseline: 1.0446x; 1.0446x over previous
"""BasisExpansionLayer Trainium2 kernel.

Full input x: [256, 512] f32. Full output: [256, 512 + 512*512 + 512] f32
laid out as [sin(x) | (x_i * x_j for the cartesian i,j grid) | x].

Sharding: the outer-product index i (512 values) is split across the 8
cores, 64 i-values each.  Every core holds the full batch (256 rows ->
2 x 128 SBUF partitions), so the DVE/ACT engines run with all 128 lanes
busy and both multiply operands are direct slices of the loaded x tiles
(no transposes, no cross-partition broadcasts, no communication).

Per core:
  pair_out[b, i_local*512 + j] = x[b, j] * x[b, c*64 + i_local]
  sin_out[b, i_local]          = sin(x[b, c*64 + i_local])
  id_out[b, i_local]           = x[b, c*64 + i_local]
The host reassembles the full [256, 263168] output from the 8 shards.
"""

import numpy as np

B = 256
D = 512
M = 8            # cores
IPC = D // M     # 64 i-values per core

_CACHE = {}

# sin(x) = y * p(y^2), y = x - round(x/2pi)*2pi (Cody-Waite), |y| <= pi.
# p coeffs: IRLS-minimax fit, end-to-end fp32 max abs err 5.3e-7.
SIN_COEFFS = [
    1.0,
    -0.166666641831398,
    0.00833331048488617,
    -0.0001984015543712303,
    2.752945647443994e-06,
    -2.467699466990325e-08,
    1.345159122978501e-10,
]
INV2PI = 0.15915494309189535
MAGIC = 12582912.0  # 1.5 * 2**23: fp32 round-to-nearest via add/sub
TWOPI_HI = 6.28125
TWOPI_LO = 0.0019353071795864769

# knobs: warm_plan = small ramp store tiles (own pool, own slot size),
# tile_plan = steady-state store-tile sizes, act_every = every
# act_every-th multiply goes to the scalar engine.
DEFAULT_CFG = dict(
    warm_plan=(2, 4, 6, 10),  # ramp store tiles for half 0 (own small pool)
    tile_plan=(16, 26),       # steady-state store tiles for half 0
    tile_plan2=(32, 32),      # half 1: pipeline already full, no ramp
    act_every=4,
    bufs=2,
    warm_bufs=3,
    repeat=1,
    sin_pos=4,       # emit the sin chain after this many pair tiles
    split_load=True,  # issue half-1 loads on the ACT HWDGE ring
    sin_from_sbuf=True,  # build the sin input by DVE copy, not DMA reload
    # software-pipelined x loads: the next repeat's loads issue after the
    # 4th store tile, so the DMA queue never drains at a rep boundary
    # (measured -11us/rep on HW; inert at repeat=1).
    xbufs=2,
    prefetch=4,
)


def _build_nc(cfg=None):
    import concourse.bass as bass  # noqa: F401
    import concourse.mybir as mybir
    import concourse.tile as tile
    from concourse import bacc

    cfg = {**DEFAULT_CFG, **(cfg or {})}
    warm_plan = tuple(cfg.get("warm_plan") or ())
    tile_plan = tuple(cfg["tile_plan"])
    tile_plan2 = tuple(cfg.get("tile_plan2") or ())
    act_every = cfg["act_every"]
    bufs = cfg["bufs"]
    warm_bufs = cfg.get("warm_bufs", 3)
    repeat = cfg["repeat"]
    sin_pos = cfg.get("sin_pos", 1 if cfg.get("sin_late") else 0)
    split_load = cfg.get("split_load", False)
    sin_from_sbuf = cfg.get("sin_from_sbuf", False)
    store_ring = cfg.get("store_ring", "sp")  # "sp" | "alt"
    warm_alt = cfg.get("warm_alt", False)  # whole warm tiles alternate DVE/ACT
    # multi-queue store distribution (None => legacy store_ring behavior):
    #   rings: tuple of ring names for pair stores, from {"sp","act","pool"}
    #   ring_assign: "rr" (round-robin by tile) | "bal" (greedy byte balance)
    #                | "csp" (split each tile's columns across all rings)
    rings = cfg.get("rings")
    ring_assign = cfg.get("ring_assign", "rr")
    load_eng = cfg.get("load_eng")  # None => legacy split_load | "sp"|"act"|"pool"
    sid_eng = cfg.get("sid_eng", "sp")
    xbufs = cfg.get("xbufs", 1)
    prefetch = cfg.get("prefetch")  # store-tile index at which to preload x
    # per-half store plans: (size, use_warm_pool) lists
    plan_h0 = [(g, True) for g in warm_plan] + [(g, False) for g in tile_plan]
    plan_h1 = (
        [(g, False) for g in tile_plan2] if tile_plan2 else list(plan_h0)
    )
    assert sum(g for g, _ in plan_h0) == IPC, plan_h0
    assert sum(g for g, _ in plan_h1) == IPC, plan_h1

    f32 = mybir.dt.float32
    nc = bacc.Bacc("TRN2", target_bir_lowering=False, debug=False, num_devices=M)

    x = nc.dram_tensor("x", [B, D], f32, kind="ExternalInput")
    xs = nc.dram_tensor("xs", [B, IPC], f32, kind="ExternalInput")
    # sin and identity merged into one [B, 2*IPC] tensor: per-partition DMA
    # runs of 512 B (the SDMA line-rate minimum) instead of 2x256 B RMW.
    sid_out = nc.dram_tensor("sid_out", [B, 2 * IPC], f32, kind="ExternalOutput")
    pair_out = nc.dram_tensor("pair_out", [B, IPC * D], f32, kind="ExternalOutput")

    def _eng(name):
        return {"sp": nc.sync, "act": nc.scalar, "pool": nc.gpsimd}[name]

    # static ring assignment for "bal": greedy least-loaded by i-count
    ring_of = []
    if rings is not None and ring_assign == "bal":
        loadb = [0] * len(rings)
        for g, _ in plan_h0 + plan_h1:
            j = min(range(len(rings)), key=lambda r: loadb[r])
            ring_of.append(j)
            loadb[j] += g

    with tile.TileContext(nc) as tc:
        with (
            tc.tile_pool(name="xp", bufs=xbufs) as xpool,
            tc.tile_pool(name="sp", bufs=1) as spool,
            tc.tile_pool(name="wp", bufs=warm_bufs) as wpool,
            tc.tile_pool(name="op", bufs=bufs) as opool,
        ):
            alu = mybir.AluOpType

            def emit_loads():
                xt_, xst_ = [], []
                for h in range(2):
                    rows = slice(h * 128, (h + 1) * 128)
                    # half-1 loads go out on the ACT HWDGE ring so both
                    # rings generate descriptors in parallel at t=0.
                    if load_eng is not None:
                        dma_eng = _eng(load_eng)
                    else:
                        dma_eng = nc.scalar if (split_load and h == 1) else nc.sync
                    ts = xpool.tile([128, IPC], f32, tag=f"xs{h}")
                    dma_eng.dma_start(ts[:], xs[rows, :])
                    xst_.append(ts)
                    t = xpool.tile([128, D], f32, tag=f"x{h}")
                    dma_eng.dma_start(t[:], x[rows, :])
                    xt_.append(t)
                return xt_, xst_

            nxt = None
            for _rep in range(repeat):
                # software-pipelined x loads: with prefetch set, rep k+1's
                # loads were already issued mid-way through rep k's store
                # stream, so the rep boundary never drains the DMA queue.
                xt, xst = emit_loads() if nxt is None else nxt
                nxt = None

                def emit_sin():
                    # accurate sin via odd polynomial on DVE: both batch
                    # halves side by side in one [128, 2*IPC] tile.
                    W = 2 * IPC
                    xsin = spool.tile([128, W], f32, tag="xsin")
                    for h in range(2):
                        if sin_from_sbuf:
                            nc.vector.tensor_copy(
                                xsin[:, h * IPC : (h + 1) * IPC], xst[h][:]
                            )
                        else:
                            nc.sync.dma_start(
                                xsin[:, h * IPC : (h + 1) * IPC],
                                xs[h * 128 : (h + 1) * 128, :],
                            )
                    tt = spool.tile([128, W], f32, tag="t")
                    # t = x*inv2pi + magic ; k = t - magic (round-to-nearest)
                    nc.vector.tensor_scalar(
                        out=tt[:], in0=xsin[:], scalar1=INV2PI, scalar2=MAGIC,
                        op0=alu.mult, op1=alu.add,
                    )
                    kk = spool.tile([128, W], f32, tag="k")
                    nc.vector.tensor_scalar_sub(kk[:], tt[:], MAGIC)
                    # y = (x - k*2pi_hi) - k*2pi_lo
                    kh = spool.tile([128, W], f32, tag="kh")
                    nc.vector.tensor_scalar_mul(kh[:], kk[:], TWOPI_HI)
                    yy = spool.tile([128, W], f32, tag="y")
                    nc.vector.tensor_sub(yy[:], xsin[:], kh[:])
                    kl = spool.tile([128, W], f32, tag="kl")
                    nc.vector.tensor_scalar_mul(kl[:], kk[:], TWOPI_LO)
                    nc.vector.tensor_sub(yy[:], yy[:], kl[:])
                    uu = spool.tile([128, W], f32, tag="u")
                    nc.vector.tensor_mul(uu[:], yy[:], yy[:])
                    # Horner: p = (((c6*u + c5)*u + c4)...)*u + c0
                    pp = spool.tile([128, W], f32, tag="p")
                    nc.vector.tensor_scalar(
                        out=pp[:], in0=uu[:], scalar1=SIN_COEFFS[6],
                        scalar2=SIN_COEFFS[5], op0=alu.mult, op1=alu.add,
                    )
                    for cidx in (4, 3, 2, 1, 0):
                        nc.vector.tensor_mul(pp[:], pp[:], uu[:])
                        nc.vector.tensor_scalar_add(
                            pp[:], pp[:], SIN_COEFFS[cidx]
                        )
                    for h in range(2):
                        rows = slice(h * 128, (h + 1) * 128)
                        sid = spool.tile([128, W], f32, tag=f"sid{h}")
                        sl = slice(h * IPC, (h + 1) * IPC)
                        # final Horner multiply lands directly in the merged
                        # tile; identity columns are a DVE copy of xs.
                        nc.vector.tensor_mul(sid[:, 0:IPC], pp[:, sl], yy[:, sl])
                        nc.vector.tensor_copy(sid[:, IPC : 2 * IPC], xst[h][:])
                        _eng(sid_eng).dma_start(sid_out[rows, :], sid[:])

                if sin_pos == 0:
                    emit_sin()

                # pair part: out[p, k*512 + j] = x[p, j] * x[p, i]
                n_op = 0
                n_tile = 0
                for h in range(2):
                    rows = slice(h * 128, (h + 1) * 128)
                    i0 = 0
                    for g_sz, warm in (plan_h0 if h == 0 else plan_h1):
                        pool = wpool if warm else opool
                        ot = pool.tile(
                            [128, g_sz * D], f32, tag="warm" if warm else "out"
                        )
                        for k in range(g_sz):
                            i = i0 + k
                            dst = ot[:, k * D : (k + 1) * D]
                            scal = xst[h][:, i : i + 1]
                            if warm_alt and warm:
                                on_act = n_tile % 2 == 1
                            else:
                                on_act = n_op % act_every == act_every - 1
                            if on_act:
                                # ACT: out = in * scale (activation Copy)
                                nc.scalar.mul(dst, xt[h][:], scal)
                            else:
                                nc.vector.tensor_scalar_mul(dst, xt[h][:], scal)
                            n_op += 1
                        if rings is None:
                            st_eng = (
                                nc.scalar
                                if (store_ring == "alt" and n_tile % 2 == 1)
                                else nc.sync
                            )
                            st_eng.dma_start(
                                pair_out[rows, i0 * D : (i0 + g_sz) * D], ot[:]
                            )
                        elif ring_assign == "csp":
                            # split this tile's columns across all rings
                            nr = len(rings)
                            c0 = 0
                            for r in range(nr):
                                gc = g_sz // nr + (1 if r < g_sz % nr else 0)
                                if gc == 0:
                                    continue
                                _eng(rings[r]).dma_start(
                                    pair_out[
                                        rows, (i0 + c0) * D : (i0 + c0 + gc) * D
                                    ],
                                    ot[:, c0 * D : (c0 + gc) * D],
                                )
                                c0 += gc
                        else:
                            if ring_assign == "bal":
                                rn = rings[ring_of[n_tile]]
                            else:  # "rr"
                                rn = rings[n_tile % len(rings)]
                            _eng(rn).dma_start(
                                pair_out[rows, i0 * D : (i0 + g_sz) * D], ot[:]
                            )
                        i0 += g_sz
                        n_tile += 1
                        if n_tile == sin_pos:
                            emit_sin()
                        if (
                            prefetch is not None
                            and n_tile == prefetch
                            and _rep + 1 < repeat
                        ):
                            nxt = emit_loads()
    nc.compile()
    return nc


def _get_nc(cfg=None):
    key = repr(cfg)
    if key not in _CACHE:
        _CACHE[key] = _build_nc(cfg)
    return _CACHE[key]


def _in_maps(x):
    x = np.ascontiguousarray(np.asarray(x, dtype=np.float32))
    assert x.shape == (B, D)
    return [
        {
            "x": x,
            "xs": np.ascontiguousarray(x[:, c * IPC : (c + 1) * IPC]),
        }
        for c in range(M)
    ]


def _get_exec(cfg=None):
    """Build the 8-core sharded PJRT callable once per process.

    Mirrors bass2jax.run_bass_via_pjrt's multi-core path, but caches the
    jitted executable: loading/executing a second NEFF in the same process
    can wedge the exec unit, while re-executing one cached executable with
    donated output buffers is reliable.
    """
    key = ("exec", repr(cfg))
    if key in _CACHE:
        return _CACHE[key]

    import jax
    from jax.sharding import Mesh, PartitionSpec
    from jax.experimental.shard_map import shard_map
    import concourse.mybir as mybir
    from concourse import bass2jax

    nc = _get_nc(cfg)
    bass2jax.install_neuronx_cc_hook()

    partition_name = nc.partition_id_tensor.name if nc.partition_id_tensor else None
    in_names, out_names, out_avals, out_shapes = [], [], [], []
    for alloc in nc.m.functions[0].allocations:
        if not isinstance(alloc, mybir.MemoryLocationSet):
            continue
        name = alloc.memorylocations[0].name
        if alloc.kind == "ExternalInput":
            if name != partition_name:
                in_names.append(name)
        elif alloc.kind == "ExternalOutput":
            shape = tuple(alloc.tensor_shape)
            dtype = mybir.dt.np(alloc.dtype)
            out_names.append(name)
            out_avals.append(jax.core.ShapedArray(shape, dtype))
            out_shapes.append((shape, dtype))
    n_params = len(in_names)
    n_outs = len(out_avals)
    all_in_names = list(in_names) + list(out_names)
    if partition_name is not None:
        all_in_names.append(partition_name)

    def _body(*args):
        operands = list(args)
        if partition_name is not None:
            operands.append(bass2jax.partition_id_tensor())
        return tuple(
            bass2jax._bass_exec_p.bind(
                *operands,
                out_avals=tuple(out_avals),
                in_names=tuple(all_in_names),
                out_names=tuple(out_names),
                lowering_input_output_aliases=(),
                sim_require_finite=True,
                sim_require_nnan=True,
                nc=nc,
            )
        )

    devices = jax.devices()[:M]
    assert len(devices) == M, f"need {M} NeuronCores, found {len(devices)}"
    mesh = Mesh(np.asarray(devices), ("core",))
    in_specs = (PartitionSpec("core"),) * (n_params + n_outs)
    out_specs = (PartitionSpec("core"),) * n_outs
    donate = tuple(range(n_params, n_params + n_outs))
    sharded = jax.jit(
        shard_map(_body, mesh=mesh, in_specs=in_specs, out_specs=out_specs,
                  check_rep=False),
        donate_argnums=donate,
        keep_unused=True,
    )

    def run(in_maps):
        concat_in = [
            np.concatenate([np.asarray(in_maps[c][n]) for c in range(M)], axis=0)
            for n in in_names
        ]
        concat_zeros = [
            np.zeros((M * s[0], *s[1:]), dt) for s, dt in out_shapes
        ]
        outs = sharded(*concat_in, *concat_zeros)
        return [
            {
                name: np.asarray(outs[i]).reshape(M, *out_shapes[i][0])[c]
                for i, name in enumerate(out_names)
            }
            for c in range(M)
        ]

    _CACHE[key] = run
    return run


def _run(x, cfg=None):
    from concourse._compat import axon_active

    if axon_active():
        return _get_exec(cfg)(_in_maps(x))
    # native NRT path (no axon): run_bass_kernel_spmd handles the NEFF
    # load/exec/unload lifecycle per call.
    from concourse import bass_utils

    res = bass_utils.run_bass_kernel_spmd(
        _get_nc(cfg), _in_maps(x), core_ids=list(range(M))
    )
    return res.results


def kernel(**inputs):
    results = _run(inputs["x"])
    out = np.empty((B, 2 * D + D * D), dtype=np.float32)
    for c in range(M):
        r = results[c]
        out[:, c * IPC : (c + 1) * IPC] = r["sid_out"][:, :IPC]
        out[:, D + c * IPC * D : D + (c + 1) * IPC * D] = r["pair_out"]
        out[:, D + D * D + c * IPC : D + D * D + (c + 1) * IPC] = r["sid_out"][:, IPC:]
    return out

